# revision 24
# baseline (speedup 1.0000x reference)
"""Multi-head cross-attention (MHAForCrossFusion) on 8 Trainium2 cores.

Strategy v3: batch x head sharding. Core c owns batch c//4 (2048 tokens)
and 4 of the 16 heads (head-group c%4, CW=256 projection features).
q/k/v are sliced per batch on the host (halves input DMA vs replication);
each core emits ONE normalized bf16 partial [2048, 1024] (its 4 heads'
contribution to the output projection); the host sums 4 partials per
batch and adds bo.

Key structural changes vs the 248us baseline (2 heads x 2 batches,
host-side normalize, per-head partial outputs):
 - on-device softmax normalize: ctx rows carry the denominator (65th
   column of ones in the augmented V, as before), reciprocal via DVE
   reciprocal_approx_fast straight from PSUM, broadcast across
   partitions on the otherwise-idle GPSIMD, and a fused
   tensor_tensor multiply during the ctx PSUM->SBUF copy. Output DMA
   drops from 16MB (2 per-head partials) to 4MB.
 - V projected TOKEN-major directly (lhsT = transposed input tile,
   rhs = Wv slice): eliminates all PE transposes and the vmF staging.
 - 4 heads run as two sequential 2-head passes over the same
   replicated-in-SBUF qm/km/vma activations, reusing the baseline's
   proven PSUM budget (scores 2x[128,1024] + ctx 2x[65,512] + w 2x).
 - out-projection contracts both heads of a pass in one K=128 matmul;
   pass 1 stages to an SBUF accumulator, pass 2 adds and ships.
"""

import numpy as np
from ml_dtypes import bfloat16

import concourse.bass as bass
import concourse.mybir as mybir
import concourse.tile as tile
from concourse import bass_utils

N_CORES = 8
B, L, D = 2, 2048, 1024
NH, HD = 16, 64
HPC = 4  # heads per core
CW = HPC * HD  # 256 features per core
DC = D // 128  # 8 contraction tiles for the projections
T = L  # per-core tokens (one batch)
NT = T // 512  # 4 token tiles
NBLK = T // 128  # 16 token blocks / key tiles
NCHUNK = T // 512  # 4 query chunks
NPAIR = NBLK // 2  # 8 key-tile pairs per chunk
SCALE = 1.0 / np.sqrt(HD)

F32 = mybir.dt.float32
BF16 = mybir.dt.bfloat16


def _split_matmul_waits(nc):
    """fp32/fp32r matmuls lower to a self-loading LDW whose ISA struct has a
    single sem-wait slot (HWDGE DMA likewise); walrus rejects >1 wait. Move
    extra waits onto same-engine NoOps inserted right before the matmul
    (program order on the sequencer preserves the happens-before)."""
    for f in nc.m.functions:
        for bb in f.blocks:
            insts = list(bb.instructions)
            out = []
            for inst in insts:
                si = inst.sync_info
                if si is not None and len(si.on_wait) > 1:
                    for w in si.on_wait[:-1]:
                        nop = mybir.InstNoOp(
                            name=nc.get_next_instruction_name(),
                            ins=[],
                            outs=[],
                            engine=inst.engine,
                            bass_nofuse=True,
                        )
                        nop.sync_info = mybir.SyncInfo(on_wait=[w], on_update=[])
                        out.append(nop)
                    inst.sync_info = mybir.SyncInfo(
                        on_wait=[si.on_wait[-1]], on_update=si.on_update
                    )
                out.append(inst)
            if len(out) != len(insts):
                bb.instructions = out
    return nc


def build_nc(dbg=False):
    nc = bass.Bass("TRN2", target_bir_lowering=False, debug=False)
    dbg_t = {}
    if dbg:
        for nm, shp in (
            ("qm_d", [128, 2 * T]), ("km_d", [128, 2 * T]),
            ("vma_d", [128, NBLK * HPC * 66]), ("ctxn_d", [128, 2 * T]),
            ("rr_d", [2, NCHUNK * 2 * 512]), ("acc_d", [128, NBLK * D]),
        ):
            dbg_t[nm] = nc.dram_tensor(nm, shp, BF16, kind="ExternalOutput").ap()

    # host-retiled per-batch inputs: [NT tiles, 128 partitions, DC*512] —
    # each (tile, partition) row is 8KB contiguous
    qT = nc.dram_tensor("qT", [NT, 128, DC * 512], BF16, kind="ExternalInput").ap()
    kT = nc.dram_tensor("kT", [NT, 128, DC * 512], BF16, kind="ExternalInput").ap()
    vT = nc.dram_tensor("vT", [NT, 128, DC * 512], BF16, kind="ExternalInput").ap()
    # weights host-swizzled to [128, DC*CW]
    wq = nc.dram_tensor("wq", [128, DC * CW], BF16, kind="ExternalInput").ap()
    wk = nc.dram_tensor("wk", [128, DC * CW], BF16, kind="ExternalInput").ap()
    wv = nc.dram_tensor("wv", [128, DC * CW], BF16, kind="ExternalInput").ap()
    # Wo.T head-slice, partition-major: [128, 2 (head-pair), D]
    wot = nc.dram_tensor("wot", [128, 2 * D], BF16, kind="ExternalInput").ap()
    bq = nc.dram_tensor("bq", [128, 2], F32, kind="ExternalInput").ap()
    bk = nc.dram_tensor("bk", [128, 2], F32, kind="ExternalInput").ap()
    bv = nc.dram_tensor("bv", [1, CW], F32, kind="ExternalInput").ap()
    out_p = nc.dram_tensor("out_p", [T, D], BF16, kind="ExternalOutput").ap()

    with tile.TileContext(nc) as tc:
        with (
            tc.tile_pool(name="singles", bufs=1) as singles,
            tc.tile_pool(name="acts", bufs=1) as acts,
            tc.tile_pool(name="slab", bufs=8) as slab_pool,
            tc.tile_pool(name="slabv", bufs=3) as slabv_pool,
            tc.tile_pool(name="es", bufs=4) as es_pool,
            tc.tile_pool(name="small", bufs=1) as small,
            tc.tile_pool(name="stg", bufs=2) as stg_pool,
            tc.tile_pool(name="ob", bufs=3) as ob_pool,
            tc.tile_pool(name="pp_sp", bufs=1, space="PSUM") as pp_sp,
            tc.tile_pool(name="pp_ctx", bufs=1, space="PSUM") as pp_ctx,
            tc.tile_pool(name="pp_w", bufs=2, space="PSUM") as pp_w,
        ):
            # ---- weights / biases to SBUF (wot deferred: needed late) ----
            w_sb = {}
            for name, dram in (("wk", wk), ("wq", wq), ("wv", wv)):
                w = singles.tile([128, DC, CW], BF16, name=name + "_sb")
                nc.sync.dma_start(w.rearrange("p c h -> p (c h)"), dram)
                w_sb[name] = w
            bq_sb = singles.tile([128, 2], F32, name="bq_sb")
            nc.sync.dma_start(bq_sb, bq)
            bk_sb = singles.tile([128, 2], F32, name="bk_sb")
            nc.sync.dma_start(bk_sb, bk)
            bv_row = singles.tile([1, CW], BF16, name="bv_row")
            bv_f32 = singles.tile([1, CW], F32, name="bv_f32")
            nc.sync.dma_start(bv_f32, bv)
            nc.vector.tensor_copy(bv_row, bv_f32)
            ones1 = singles.tile([1, 512], BF16, name="ones1")
            nc.vector.memset(ones1, 1.0)
            # HAM warm-up: cheap K=1 matmuls keep the PE busy while the
            # first input tiles stream in, so real matmuls start at 2.4GHz
            warm_ps = pp_w.tile([128, 512], F32, tag="w", name="warm_ps")
            for _ in range(16):
                nc.tensor.matmul(warm_ps[0:1, :], lhsT=ones1[:, 0:1], rhs=ones1)
            # bvb = ones ⊗ bv (partition-broadcast via K=1 matmul)
            bvb = singles.tile([128, CW], F32, name="bvb")
            bvp = pp_w.tile([128, 512], F32, tag="w", name="bvp")
            nc.tensor.matmul(bvp[:, 0:CW], lhsT=ones1[:, 0:128], rhs=bv_row)
            nc.vector.tensor_copy(bvb, bvp[:, 0:CW])

            # ---- activations ----
            qm = acts.tile([128, 2, T], BF16)  # [feat-in-pair, pair, token]
            km = acts.tile([128, 2, T], BF16)
            # vma: [token%128, block, head-group*65 + (feat|one)]
            vma = acts.tile([128, NBLK, HPC * 66], BF16)  # 64 feats | one | pad
            ctxn = acts.tile([128, 2, T], BF16)  # normalized ctx, feat-major
            out_acc = acts.tile([128, NBLK, D], BF16)  # pass-1 partial

            # ones columns of the augmented V (col 64 per head group)
            nc.vector.memset(
                vma.rearrange("p t (g c) -> p t g c", c=66)[:, :, :, 64:66], 1.0
            )

            # ---- input streaming ----
            dma_order = [
                ("wk", 0), ("wq", 0), ("wv", 0), ("wk", 1), ("wv", 1),
                ("wk", 2), ("wv", 2), ("wk", 3), ("wv", 3), ("wq", 1),
                ("wq", 2), ("wq", 3),
            ]
            dram_of = {"wk": kT, "wq": qT, "wv": vT}
            wot_sb = singles.tile([128, 2, D], BF16)
            xt = {}
            for name, ti in dma_order:
                pool = slabv_pool if name == "wv" else slab_pool
                t = pool.tile(
                    [128, DC, 512], BF16, tag="xt" + ("v" if name == "wv" else ""),
                    name=f"xt_{name}_{ti}",
                )
                nc.sync.dma_start(t.rearrange("p c t -> p (c t)"), dram_of[name][ti])
                xt[(name, ti)] = t
                if (name, ti) == ("wv", 0):
                    nc.sync.dma_start(wot_sb.rearrange("p h d -> p (h d)"), wot)

            def emit_proj(name, ti, hp):
                """q/k projection of one 512-token tile, feature-major,
                one head-pair (128 output features)."""
                dstf = qm if name == "wq" else km
                b_sb = bq_sb if name == "wq" else bk_sb
                ps = pp_w.tile([128, 512], F32, tag="w", name="ps")
                for dc in range(DC):
                    nc.tensor.matmul(
                        ps,
                        lhsT=w_sb[name][:, dc, hp * 128 : (hp + 1) * 128],
                        rhs=xt[(name, ti)][:, dc, :],
                        start=(dc == 0),
                        stop=(dc == DC - 1),
                    )
                nc.vector.tensor_scalar_add(
                    dstf[:, hp, ti * 512 : (ti + 1) * 512], ps, b_sb[:, hp : hp + 1]
                )

            def emit_vproj(blk):
                """V projection of one 128-token block, token-major, all 4
                heads (256 features), fused bias add into the vma copy."""
                ti, off = blk // 4, (blk % 4) * 128
                ps = pp_w.tile([128, 512], F32, tag="w", name="vps")
                for dc in range(DC):
                    nc.tensor.matmul(
                        ps[:, 0:CW],
                        lhsT=xt[("wv", ti)][:, dc, off : off + 128],
                        rhs=w_sb["wv"][:, dc, :],
                        start=(dc == 0),
                        stop=(dc == DC - 1),
                    )
                nc.vector.tensor_tensor(
                    vma.rearrange("p t (g c) -> p t g c", c=66)[:, blk, :, 0:64],
                    ps[:, 0 : CW].rearrange("p (g c) -> p g c", c=64),
                    bvb.rearrange("p (g c) -> p g c", c=64),
                    mybir.AluOpType.add,
                )

            # ---- attention pipeline ----
            def emit_scores(hp, c, p, state):
                ls = slice(c * 512, (c + 1) * 512)
                sp = [
                    pp_sp.tile([128, 1024], F32, tag=f"sp{h}", name=f"sp{h}")
                    for h in range(2)
                ]
                es = [
                    es_pool.tile([128, 1024], BF16, tag=f"es{h}", name=f"es{h}")
                    for h in range(2)
                ]
                for h in range(2):
                    hs = slice(h * 64, (h + 1) * 64)
                    for i in range(2):
                        kt = p * 2 + i
                        ks = slice(kt * 128, (kt + 1) * 128)
                        nc.tensor.matmul(
                            sp[h][:, i * 512 : (i + 1) * 512],
                            lhsT=km[hs, hp, ks],
                            rhs=qm[hs, hp, ls],
                            tile_position=(h * 64, 0),
                        )
                    nc.scalar.activation(
                        es[h], sp[h], mybir.ActivationFunctionType.Exp, scale=SCALE
                    )
                state["es"][(hp, c, p)] = es

            def emit_ctx(hp, c, p, state):
                es = state["es"].pop((hp, c, p))
                if p == 0:
                    state["ctx"][(hp, c)] = [
                        pp_ctx.tile([128, 512], F32, tag=f"ctx{h}", name=f"ctx{h}")
                        for h in range(2)
                    ]
                ctx = state["ctx"][(hp, c)]
                for i in range(2):
                    kt = p * 2 + i
                    for h in range(2):
                        g = hp * 2 + h
                        nc.tensor.matmul(
                            ctx[h][0:65, :],
                            lhsT=vma[:, kt, g * 66 : g * 66 + 65],
                            rhs=es[h][:, i * 512 : (i + 1) * 512],
                            start=(p == 0 and i == 0),
                            stop=(p == NPAIR - 1 and i == 1),
                        )

            def emit_ctxcopy(hp, c, state):
                """stage ctx+den rows to SBUF with two plain copies so the
                ctx PSUM banks free immediately; the normalize chain then
                runs entirely off SBUF without stalling the next chunk."""
                ctx = state["ctx"][(hp, c)]
                stg = stg_pool.tile([65, 2, 512], BF16, tag="stg", name="stg")
                for h in range(2):
                    nc.vector.tensor_copy(stg[:, h, :], ctx[h][0:65, :])
                state["stg"][(hp, c)] = stg

            def emit_norm(hp, c, state):
                """1/den via ACT ln -> exp(-x) (both in the natural_log_exp
                table set with the scores exp: no table switches), both heads
                in one FD=1024 op; partition-broadcast via K=1 matmuls, then
                the divide is a DVE multiply into feature-major ctxn."""
                stg = state["stg"].pop((hp, c))
                lr = small.tile([1, 1024], F32, tag="lr", name="lr")
                nc.scalar.activation(
                    lr, stg[64:65, :, :], mybir.ActivationFunctionType.Ln
                )
                rr = small.tile([1, 2, 512], BF16, tag="rr", name="rr")
                nc.scalar.activation(
                    rr, lr, mybir.ActivationFunctionType.Exp, scale=-1.0
                )
                for h in range(2):
                    rbp = pp_w.tile([128, 512], F32, tag="w", name="rbp")
                    nc.tensor.matmul(
                        rbp[0:64, :], lhsT=ones1[:, 0:64], rhs=rr[:, h, :]
                    )
                    rbs = small.tile([64, 512], F32, tag=f"rb{h}", name="rbs")
                    nc.vector.tensor_copy(rbs, rbp[0:64, :])
                    nc.vector.tensor_tensor(
                        ctxn[h * 64 : (h + 1) * 64, hp, c * 512 : (c + 1) * 512],
                        stg[0:64, h, :],
                        rbs,
                        mybir.AluOpType.mult,
                    )
                    if dbg:
                        off = (c * 2 + h) * 512
                        nc.sync.dma_start(
                            dbg_t["rr_d"][hp : hp + 1, off : off + 512], rr[:, h, :]
                        )

            def emit_outproj(hp, c, tt):
                """one 128-token block: K=128 matmul over the pass's 2 heads;
                pass 0 stages to out_acc, pass 1 adds and ships."""
                blk = c * 4 + tt
                t0 = blk * 128
                for eh in range(2):
                    ps = pp_w.tile([128, 512], F32, tag="w", name="po")
                    nc.tensor.matmul(
                        ps,
                        lhsT=ctxn[:, hp, t0 : t0 + 128],
                        rhs=wot_sb[:, hp, eh * 512 : (eh + 1) * 512],
                    )
                    if hp == 0:
                        nc.vector.tensor_copy(
                            out_acc[:, blk, eh * 512 : (eh + 1) * 512], ps
                        )
                    else:
                        ob = ob_pool.tile([128, 512], BF16, tag="ob", name="ob")
                        nc.vector.tensor_tensor(
                            ob, ps, out_acc[:, blk, eh * 512 : (eh + 1) * 512],
                            mybir.AluOpType.add,
                        )
                        nc.sync.dma_start(
                            out_p[t0 : t0 + 128, eh * 512 : (eh + 1) * 512], ob
                        )

            # ---- schedule ----
            # prologue: everything chunk (0,0) needs, plus all km01/qm01-t1
            # (DMA-paced anyway); vma blocks 4-15 ride the first steps at
            # their kt deadlines
            for ti in range(4):
                emit_proj("wk", ti, 0)
                if ti < 2:
                    emit_proj("wq", ti, 0)
            for blk in range(4):
                emit_vproj(blk)

            P = "proj"
            V = "vproj"
            O = "outproj"
            fill = {}

            def add(st, unit):
                fill.setdefault(st, []).append(unit)

            for blk in range(4, 16):  # deadline: step blk//2 - 1 (ctx lag 3)
                add(max(0, blk // 2 - 2), (V, blk))
            add(9, (P, "wq", 2, 0))
            add(17, (P, "wq", 3, 0))
            for i, st in enumerate((20, 26, 31, 34)):
                add(st, (P, "wk", i, 1))
            for i, st in enumerate((22, 36, 42, 50)):
                add(st, (P, "wq", i, 1))
            outproj_slots = {
                (0, 0): (13, 18, 23, 27), (0, 1): (21, 25, 29, 33),
                (0, 2): (30, 35, 38, 41), (0, 3): (37, 40, 43, 46),
                (1, 0): (45, 48, 51, 54), (1, 1): (53, 56, 58, 60),
                (1, 2): (61, 62, 63, 64), (1, 3): (68, 69, 70, 71),
            }
            for (hp, c), sts in outproj_slots.items():
                for tt, st in enumerate(sts):
                    add(st, (O, hp, c, tt))

            NP_TOT = 2 * NCHUNK * NPAIR  # 64 steps

            def cp(s):
                return s // 32, (s % 32) // 8, s % 8  # hp, c, p

            state = {"es": {}, "ctx": {}, "stg": {}}
            for s in range(NP_TOT + 16):
                for unit in fill.pop(s, []):
                    if unit[0] is P:
                        _, name, ti, hp = unit
                        emit_proj(name, ti, hp)
                    elif unit[0] is V:
                        emit_vproj(unit[1])
                    else:
                        _, hp, c, tt = unit
                        emit_outproj(hp, c, tt)
                # scores first: their sp-free waits were satisfied during
                # the previous step, so the PE never idles at step entry
                if s < NP_TOT:
                    emit_scores(*cp(s), state)
                if 0 <= s - 3 < NP_TOT:
                    hp, c, p = cp(s - 3)
                    emit_ctx(hp, c, p, state)
                    if p == NPAIR - 1:
                        emit_ctxcopy(hp, c, state)
                if 0 <= s - 4 < NP_TOT:
                    hp, c, p = cp(s - 4)
                    if p == NPAIR - 1:
                        emit_norm(hp, c, state)
            assert not fill, f"unplaced fillers: {sorted(fill)}"
            if dbg:
                nc.sync.dma_start(dbg_t["qm_d"], qm.rearrange("p h t -> p (h t)"))
                nc.sync.dma_start(dbg_t["km_d"], km.rearrange("p h t -> p (h t)"))
                nc.sync.dma_start(dbg_t["vma_d"], vma.rearrange("p t c -> p (t c)"))
                nc.sync.dma_start(dbg_t["ctxn_d"], ctxn.rearrange("p h t -> p (h t)"))
                nc.sync.dma_start(dbg_t["acc_d"], out_acc.rearrange("p t c -> p (t c)"))
    return _split_matmul_waits(nc)


_NC_CACHE = None


def build_in_maps(q, k, v, Wq, bq, Wk, bk, Wv, bv, Wo, bo):
    q, k, v = (np.asarray(x, np.float32) for x in (q, k, v))

    def retile(x):
        # [T, D] -> xT [D, T] -> [NT, 128, DC*512], each (tile, partition)
        # row contiguous
        xt = x.reshape(T, D).T.reshape(DC, 128, NT, 512)
        return (
            np.ascontiguousarray(xt.transpose(2, 1, 0, 3))
            .reshape(NT, 128, DC * 512)
            .astype(bfloat16)
        )

    qTh = [retile(q[b]) for b in range(B)]
    kTh = [retile(k[b]) for b in range(B)]
    vTh = [retile(v[b]) for b in range(B)]

    def swz(W, hs):
        wt = np.asarray(W, np.float32).T[:, hs]
        return (
            wt.reshape(DC, 128, CW).transpose(1, 0, 2).reshape(128, DC * CW)
            .astype(bfloat16)
        )

    c = np.ascontiguousarray
    in_maps = []
    for ci in range(N_CORES):
        b, hg = divmod(ci, B * 2)  # b = ci // 4, hg = ci % 4
        hs = slice(hg * CW, (hg + 1) * CW)
        in_maps.append(
            {
                "qT": qTh[b],
                "kT": kTh[b],
                "vT": vTh[b],
                "wq": swz(Wq, hs),
                "wk": swz(Wk, hs),
                "wv": swz(Wv, hs),
                "wot": c(
                    np.asarray(Wo, np.float32).T[hs, :]
                    .reshape(2, 128, D).transpose(1, 0, 2).reshape(128, 2 * D)
                ).astype(bfloat16),
                "bq": c(np.asarray(bq, np.float32)[hs].reshape(2, 128).T),
                "bk": c(np.asarray(bk, np.float32)[hs].reshape(2, 128).T),
                "bv": c(np.asarray(bv, np.float32)[hs].reshape(1, CW)),
            }
        )
    return in_maps


def run(inputs, trace=False, **spmd_kwargs):
    global _NC_CACHE
    assert np.asarray(inputs["attention_mask"]).all(), "kernel assumes all-ones mask"
    if _NC_CACHE is None:
        _NC_CACHE = build_nc()
    nc = _NC_CACHE
    in_maps = build_in_maps(
        **{n: inputs[n] for n in ("q", "k", "v", "Wq", "bq", "Wk", "bk", "Wv", "bv", "Wo", "bo")}
    )
    res = bass_utils.run_bass_kernel_spmd(
        nc, in_maps, core_ids=list(range(N_CORES)), trace=trace, **spmd_kwargs
    )
    out = np.zeros((B, T, D), np.float32)
    for ci, r in enumerate(res.results):
        out[ci // 4] += np.asarray(r["out_p"], dtype=np.float32)
    out += np.asarray(inputs["bo"], np.float32)[None, None, :]
    return out, res


def kernel(q, k, v, attention_mask, Wq, bq, Wk, bk, Wv, bv, Wo, bo):
    out, _ = run(dict(q=q, k=k, v=v, attention_mask=attention_mask, Wq=Wq, bq=bq,
                      Wk=Wk, bk=bk, Wv=Wv, bv=bv, Wo=Wo, bo=bo))
    return out
